# revision 8
# baseline (speedup 1.0000x reference)
"""Trainium2 Bass kernel for the word2vec-style embedding lookup problem.

reference:
    inputs = paragraph_matrix[doc_ids] + sum(word_matrix[context_ids], axis=1)
    out_cols = outputs[:, sample_ids].transpose(1, 0, 2)
    return einsum("bd,bds->bs", inputs, out_cols)

Strategy: data-parallel over the batch dim across 8 NeuronCores. The host
dedups each core's needed table rows into a per-core compact fp16 table
(standard table-sharding: rows this core touches, each exactly once):

  ctab[0:18432)      unique doc/ctx rows   (A window, int16-addressable)
  ctab[18432:51200)  unique sample columns (B window, exactly 32768 rows)

The device then does tile-aligned Q7 `dma_gather`s straight into batch
layout (stream position (k*128+p) -> batch element p of the tile):

  A: 2 gathers x 9216 rows -> [128, 8, 9, 128]; tree-add the 9 rows
     (packed-fp16 2x DVE mode) -> inputs[128, 16, 128] in SBUF
  B: 4 gathers x 8192 rows -> [128, 4, 16, 128]; mul by broadcast inputs
     + halving-add tree over d (2x mode) -> res[128, 16, 16]

fp16 everywhere (2e-2 rel-err budget; fp16 keeps us ~1e-3): 256B rows
halve HBM traffic and enable the 2x DVE modes. 51200 gathered rows/core
vs 108032 for the chunk-sorted two-stage design, no intermediate DRAM
round trip, no stream-order unpermute on the host.
"""

import numpy as np

import concourse.mybir as mybir
from concourse.bacc import Bacc
from concourse.tile import TileContext

# Problem constants (hardcoded per harness contract).
VEC = 128
N_DOCS = 100000
N_WORDS = 100000
B = 16384
CTX = 8
NS = 16
N_CORES = 8
P = 128

B_CORE = B // N_CORES            # 2048
N_TILES = B_CORE // P            # 16

# A-phase: 2 gathers of 8 tiles x 9 rows; B-phase: 4 gathers of 4 tiles x 16.
A_TPG = 8
A_NG = N_TILES // A_TPG          # 2
A_ROWS = A_TPG * 9 * P           # 9216
B_TPG = 4
B_NG = N_TILES // B_TPG          # 4
B_ROWS = B_TPG * NS * P          # 8192

N_A = B_CORE + B_CORE * CTX      # 18432 worst-case unique doc+ctx rows
N_B = B_CORE * NS                # 32768 worst-case unique sample rows
CTAB_ROWS = N_A + N_B            # 51200
IDX_COLS = (A_NG * A_ROWS + B_NG * B_ROWS) // 16  # 3200


def _wrap16(stream: np.ndarray) -> np.ndarray:
    """dma_gather index layout: j at [16k + j%16, j//16], replicated 8x."""
    assert len(stream) % 16 == 0
    arr = stream.astype(np.int16).reshape(-1, 16).T  # [16, n/16]
    return np.tile(arr, (8, 1))                      # [128, n/16]


def build_nc(queue_map=None, reps=1):
    nc = Bacc("TRN2", num_swdge_queues=4)
    f16, i16 = mybir.dt.float16, mybir.dt.int16
    ctab = nc.dram_tensor("ctab", [CTAB_ROWS, VEC], f16, kind="ExternalInput")
    idx = nc.dram_tensor("idx", [P, IDX_COLS], i16, kind="ExternalInput")
    res = nc.dram_tensor("res", [B_CORE, NS], f16, kind="ExternalOutput")

    qi = [0]

    def next_q():
        q = queue_map[qi[0] % len(queue_map)] if queue_map is not None else 0
        qi[0] += 1
        return q

    def emit_body(tc, idx_all, pools):
        a_pool, b_pool, tmp_pool, acc_pool = pools
        col = [0]

        def idx_slice(n):
            ap = idx_all[:, col[0]:col[0] + n // 16]
            col[0] += n // 16
            return ap

        inputs_all = acc_pool.tile([P, N_TILES, 1, VEC], f16, tag="inp")
        res_all = acc_pool.tile([P, N_TILES, NS], f16, tag="res")

        # ---- A: gather 9 rows per element, tree-add -> inputs ----
        for c in range(A_NG):
            ct = slice(c * A_TPG, (c + 1) * A_TPG)
            t9 = a_pool.tile([P, A_TPG * 9, VEC], f16, tag="t9")
            nc.gpsimd.dma_gather(
                t9[:, :, :],
                ctab[0:N_A, :],
                idx_slice(A_ROWS),
                A_ROWS, A_ROWS, VEC,
                queue_num=next_q(), single_packet=False,
            )
            v = t9[:, :, :].rearrange("p (t r) d -> p t r d", r=9)
            s4 = tmp_pool.tile([P, A_TPG, 4, VEC], f16, tag="s4")
            nc.vector.tensor_add(
                out=s4[:, :, :, :], in0=v[:, :, 0:4, :], in1=v[:, :, 4:8, :])
            s2 = tmp_pool.tile([P, A_TPG, 2, VEC], f16, tag="s2")
            nc.vector.tensor_add(
                out=s2[:, :, :, :], in0=s4[:, :, 0:2, :], in1=s4[:, :, 2:4, :])
            s1 = tmp_pool.tile([P, A_TPG, 1, VEC], f16, tag="s1")
            nc.vector.tensor_add(
                out=s1[:, :, :, :], in0=s2[:, :, 0:1, :], in1=s2[:, :, 1:2, :])
            nc.vector.tensor_add(
                out=inputs_all[:, ct, :, :], in0=s1[:, :, :, :],
                in1=v[:, :, 8:9, :])

        # ---- B: gather 16 sample columns per element, dot with inputs ----
        for g in range(B_NG):
            gt = slice(g * B_TPG, (g + 1) * B_TPG)
            smp = b_pool.tile([P, B_TPG * NS, VEC], f16, tag="smp")
            nc.gpsimd.dma_gather(
                smp[:, :, :],
                ctab[N_A:CTAB_ROWS, :],
                idx_slice(B_ROWS),
                B_ROWS, B_ROWS, VEC,
                queue_num=next_q(), single_packet=False,
            )
            sv = smp[:, :, :].rearrange("p (t s) d -> p t s d", s=NS)
            nc.vector.tensor_mul(
                out=sv,
                in0=sv,
                in1=inputs_all[:, gt, :, :].to_broadcast(
                    [P, B_TPG, NS, VEC]),
            )
            # Reduce over d via in-place halving adds (2x packed-fp16 mode),
            # then one small TensorReduce over the last 4 elements.
            w = VEC
            while w > 4:
                w //= 2
                nc.vector.tensor_add(
                    out=sv[:, :, :, 0:w],
                    in0=sv[:, :, :, 0:w],
                    in1=sv[:, :, :, w:2 * w],
                )
            with nc.allow_low_precision("fp16 dot, 2e-2 rel-err budget"):
                nc.vector.reduce_sum(
                    out=res_all[:, gt, :],
                    in_=sv[:, :, :, 0:4],
                    axis=mybir.AxisListType.X,
                )

        nc.sync.dma_start(
            out=res[:, :].rearrange("(t p) s -> p t s", p=P),
            in_=res_all[:, :, :],
        )

    with TileContext(nc) as tc:
        with (
            tc.tile_pool(name="idxp", bufs=1) as idx_pool,
            tc.tile_pool(name="ap", bufs=2) as a_pool,
            tc.tile_pool(name="bp", bufs=2) as b_pool,
            tc.tile_pool(name="tmp", bufs=2) as tmp_pool,
            tc.tile_pool(name="acc", bufs=1) as acc_pool,
        ):
            idx_all = idx_pool.tile([P, IDX_COLS], mybir.dt.int16)
            nc.sync.dma_start(out=idx_all[:, :], in_=idx[:, :])
            pools = (a_pool, b_pool, tmp_pool, acc_pool)
            for _rep in range(reps):
                emit_body(tc, idx_all, pools)

    nc.finalize()
    return nc


def gather_queue_map(nc):
    """Read each dma_gather's Tile-assigned DMASW lane; queue = lane % 4
    keeps every sem lane on a single SWDGE queue."""
    lanes = []
    for f in nc.m.functions:
        for blk in f.blocks:
            for ins in blk.instructions:
                if type(ins).__name__ == "InstDMAGatherAnt":
                    si = ins.sync_info
                    lane = None
                    for u in (si.on_update or []):
                        name = u.ant_name or ""
                        if name.startswith("DMASW"):
                            lane = int(name[5:].split("_")[0])
                    lanes.append((ins.name, lane))
    # instruction names I-k are in emission order; sort by numeric id
    lanes.sort(key=lambda t: int(t[0].split("-")[1]))
    return [(l % 4 if l is not None else 0) for _, l in lanes]


def build_nc_queued(reps=1):
    nc0 = build_nc(reps=reps)
    qmap = gather_queue_map(nc0)
    nc1 = build_nc(queue_map=qmap, reps=reps)
    qmap1 = gather_queue_map(nc1)
    if qmap1 != qmap:
        nc1 = build_nc(queue_map=qmap1, reps=reps)
    return nc1


def prepare_host(doc_ids, context_ids, sample_ids, paragraph_matrix,
                 word_matrix, outputs):
    doc_ids = np.asarray(doc_ids).astype(np.int64)
    context_ids = np.asarray(context_ids).astype(np.int64)
    sample_ids = np.asarray(sample_ids).astype(np.int64)
    full = np.concatenate(
        [
            np.asarray(paragraph_matrix, dtype=np.float32),
            np.asarray(word_matrix, dtype=np.float32),
            np.ascontiguousarray(np.asarray(outputs, dtype=np.float32).T),
        ],
        axis=0,
    ).astype(np.float16)

    idsA = np.concatenate(
        [doc_ids[:, None], context_ids + N_DOCS], axis=1)   # [B, 9]
    idsB = sample_ids + (N_DOCS + N_WORDS)                  # [B, 16]

    in_maps = []
    for c in range(N_CORES):
        sl = slice(c * B_CORE, (c + 1) * B_CORE)
        uqA, invA = np.unique(idsA[sl].ravel(), return_inverse=True)
        uqB, invB = np.unique(idsB[sl].ravel(), return_inverse=True)
        assert len(uqA) <= N_A and len(uqB) <= N_B
        ctab = np.zeros((CTAB_ROWS, VEC), dtype=np.float16)
        ctab[:len(uqA)] = full[uqA]
        ctab[N_A:N_A + len(uqB)] = full[uqB]
        cidA = invA.reshape(B_CORE, 9)
        cidB = invB.reshape(B_CORE, NS)

        streams = []
        for a in range(A_NG):
            blk = cidA[a * A_TPG * P:(a + 1) * A_TPG * P]
            streams.append(
                blk.reshape(A_TPG, P, 9).transpose(0, 2, 1).ravel())
        for g in range(B_NG):
            blk = cidB[g * B_TPG * P:(g + 1) * B_TPG * P]
            streams.append(
                blk.reshape(B_TPG, P, NS).transpose(0, 2, 1).ravel())
        idx = np.concatenate([_wrap16(s) for s in streams], axis=1)
        assert idx.shape == (P, IDX_COLS)
        in_maps.append({"ctab": ctab, "idx": idx})
    return in_maps


def kernel(doc_ids, context_ids, sample_ids, paragraph_matrix, word_matrix,
           outputs):
    from concourse.bass_utils import run_bass_kernel_spmd

    in_maps = prepare_host(doc_ids, context_ids, sample_ids,
                           paragraph_matrix, word_matrix, outputs)
    nc = build_nc_queued()
    out = run_bass_kernel_spmd(nc, in_maps, core_ids=list(range(N_CORES)))

    result = np.empty((B, NS), dtype=np.float32)
    for c in range(N_CORES):
        result[c * B_CORE:(c + 1) * B_CORE] = \
            out.results[c]["res"].astype(np.float32)
    return result


if __name__ == "__main__":
    pass


# revision 14
# speedup vs baseline: 7.9779x; 7.9779x over previous
"""Trainium2 Bass kernel for the word2vec-style embedding lookup problem.

reference:
    inputs = paragraph_matrix[doc_ids] + sum(word_matrix[context_ids], axis=1)
    out_cols = outputs[:, sample_ids].transpose(1, 0, 2)
    return einsum("bd,bds->bs", inputs, out_cols)

Strategy: data-parallel over the batch dim across 8 NeuronCores. The host
dedups each core's needed table rows into a per-core compact fp16 table
(standard table-sharding: rows this core touches, each exactly once):

  ctab[0:18432)      unique doc/ctx rows   (A window, int16-addressable)
  ctab[18432:51200)  unique sample columns (B window, exactly 32768 rows)

The device then does tile-aligned Q7 `dma_gather`s straight into batch
layout (stream position (k*128+p) -> batch element p of the tile):

  A: 2 gathers x 9216 rows -> [128, 8, 9, 128]; tree-add the 9 rows
     (packed-fp16 2x DVE mode) -> inputs[128, 16, 128] in SBUF
  B: 4 gathers x 8192 rows -> [128, 4, 16, 128]; mul by broadcast inputs
     + halving-add tree over d (2x mode) -> res[128, 16, 16]

fp16 everywhere (2e-2 rel-err budget; fp16 keeps us ~1e-3): 256B rows
halve HBM traffic and enable the 2x DVE modes. 51200 gathered rows/core
vs 108032 for the chunk-sorted two-stage design, no intermediate DRAM
round trip, no stream-order unpermute on the host.
"""

import numpy as np

import concourse.mybir as mybir
from concourse.bacc import Bacc
from concourse.tile import TileContext

# Problem constants (hardcoded per harness contract).
VEC = 128
N_DOCS = 100000
N_WORDS = 100000
B = 16384
CTX = 8
NS = 16
N_CORES = 8
P = 128

B_CORE = B // N_CORES            # 2048
N_TILES = B_CORE // P            # 16

# A-phase: 4 gathers of 4 tiles x 9 rows; B-phase: 8 gathers of 2 tiles x 16.
A_TPG = 4
A_NG = N_TILES // A_TPG          # 4
A_ROWS = A_TPG * 9 * P           # 4608
B_TPG = 2
B_NG = N_TILES // B_TPG          # 8
B_ROWS = B_TPG * NS * P          # 4096

N_A = B_CORE + B_CORE * CTX      # 18432 worst-case unique doc+ctx rows
N_B = B_CORE * NS                # 32768 worst-case unique sample rows
CTAB_ROWS = N_A + N_B            # 51200
IDX_COLS = (A_NG * A_ROWS + B_NG * B_ROWS) // 16  # 3200


def _wrap16(stream: np.ndarray) -> np.ndarray:
    """dma_gather index layout: j at [16k + j%16, j//16], replicated 8x."""
    assert len(stream) % 16 == 0
    arr = stream.astype(np.int16).reshape(-1, 16).T  # [16, n/16]
    return np.tile(arr, (8, 1))                      # [128, n/16]


def build_nc(queue_map=None, reps=1):
    nc = Bacc("TRN2", num_swdge_queues=4)
    f16, i16 = mybir.dt.float16, mybir.dt.int16
    ctab = nc.dram_tensor("ctab", [CTAB_ROWS, VEC], f16, kind="ExternalInput")
    idx = nc.dram_tensor("idx", [P, IDX_COLS], i16, kind="ExternalInput")
    res = nc.dram_tensor("res", [B_CORE, NS], f16, kind="ExternalOutput")

    qi = [0]

    def next_q():
        q = queue_map[qi[0] % len(queue_map)] if queue_map is not None else 0
        qi[0] += 1
        return q

    def emit_body(tc, idx_all, pools):
        a_pool, b_pool, tmp_pool, acc_pool = pools
        col = [0]

        def idx_slice(n):
            ap = idx_all[:, col[0]:col[0] + n // 16]
            col[0] += n // 16
            return ap

        inputs_all = acc_pool.tile([P, N_TILES, 1, VEC], f16, tag="inp")
        res_all = acc_pool.tile([P, N_TILES, NS], f16, tag="res")

        # ---- interleave A groups with their dependent B groups ----
        def emit_a(c):
            ct = slice(c * A_TPG, (c + 1) * A_TPG)
            t9 = a_pool.tile([P, A_TPG * 9, VEC], f16, tag="t9")
            nc.gpsimd.dma_gather(
                t9[:, :, :],
                ctab[0:N_A, :],
                idx_slice(A_ROWS),
                A_ROWS, A_ROWS, VEC,
                queue_num=next_q(), single_packet=False,
            )
            v = t9[:, :, :].rearrange("p (t r) d -> p t r d", r=9)
            s4 = tmp_pool.tile([P, A_TPG, 4, VEC], f16, tag="s4")
            nc.vector.tensor_add(
                out=s4[:, :, :, :], in0=v[:, :, 0:4, :], in1=v[:, :, 4:8, :])
            s2 = tmp_pool.tile([P, A_TPG, 2, VEC], f16, tag="s2")
            nc.vector.tensor_add(
                out=s2[:, :, :, :], in0=s4[:, :, 0:2, :], in1=s4[:, :, 2:4, :])
            s1 = tmp_pool.tile([P, A_TPG, 1, VEC], f16, tag="s1")
            nc.vector.tensor_add(
                out=s1[:, :, :, :], in0=s2[:, :, 0:1, :], in1=s2[:, :, 1:2, :])
            nc.vector.tensor_add(
                out=inputs_all[:, ct, :, :], in0=s1[:, :, :, :],
                in1=v[:, :, 8:9, :])

        def emit_b(g):
            gt = slice(g * B_TPG, (g + 1) * B_TPG)
            smp = b_pool.tile([P, B_TPG * NS, VEC], f16, tag="smp")
            nc.gpsimd.dma_gather(
                smp[:, :, :],
                ctab[N_A:CTAB_ROWS, :],
                idx_slice(B_ROWS),
                B_ROWS, B_ROWS, VEC,
                queue_num=next_q(), single_packet=False,
            )
            sv = smp[:, :, :].rearrange("p (t s) d -> p t s d", s=NS)
            nc.vector.tensor_mul(
                out=sv,
                in0=sv,
                in1=inputs_all[:, gt, :, :].to_broadcast(
                    [P, B_TPG, NS, VEC]),
            )
            # Reduce over d via in-place halving adds (2x packed-fp16 mode),
            # then one small TensorReduce over the last 4 elements.
            w = VEC
            while w > 4:
                w //= 2
                nc.vector.tensor_add(
                    out=sv[:, :, :, 0:w],
                    in0=sv[:, :, :, 0:w],
                    in1=sv[:, :, :, w:2 * w],
                )
            with nc.allow_low_precision("fp16 dot, 2e-2 rel-err budget"):
                nc.vector.reduce_sum(
                    out=res_all[:, gt, :],
                    in_=sv[:, :, :, 0:4],
                    axis=mybir.AxisListType.X,
                )

        bpg = B_NG // A_NG
        for c in range(A_NG):
            emit_a(c)
            for g in range(c * bpg, (c + 1) * bpg):
                emit_b(g)

        nc.sync.dma_start(
            out=res[:, :].rearrange("(t p) s -> p t s", p=P),
            in_=res_all[:, :, :],
        )

    with TileContext(nc) as tc:
        with (
            tc.tile_pool(name="idxp", bufs=1) as idx_pool,
            tc.tile_pool(name="ap", bufs=4) as a_pool,
            tc.tile_pool(name="bp", bufs=6) as b_pool,
            tc.tile_pool(name="tmp", bufs=4) as tmp_pool,
            tc.tile_pool(name="acc", bufs=1) as acc_pool,
        ):
            idx_all = idx_pool.tile([P, IDX_COLS], mybir.dt.int16)
            nc.sync.dma_start(out=idx_all[:, :], in_=idx[:, :])
            pools = (a_pool, b_pool, tmp_pool, acc_pool)
            for _rep in range(reps):
                emit_body(tc, idx_all, pools)

    nc.finalize()
    return nc


def gather_queue_map(nc):
    """Read each dma_gather's Tile-assigned DMASW lane; queue = lane % 4
    keeps every sem lane on a single SWDGE queue."""
    lanes = []
    for f in nc.m.functions:
        for blk in f.blocks:
            for ins in blk.instructions:
                if type(ins).__name__ == "InstDMAGatherAnt":
                    si = ins.sync_info
                    lane = None
                    for u in (si.on_update or []):
                        name = u.ant_name or ""
                        if name.startswith("DMASW"):
                            lane = int(name[5:].split("_")[0])
                    lanes.append((ins.name, lane))
    # instruction names I-k are in emission order; sort by numeric id
    lanes.sort(key=lambda t: int(t[0].split("-")[1]))
    return [(l % 4 if l is not None else 0) for _, l in lanes]


def build_nc_queued(reps=1):
    nc0 = build_nc(reps=reps)
    qmap = gather_queue_map(nc0)
    nc1 = build_nc(queue_map=qmap, reps=reps)
    qmap1 = gather_queue_map(nc1)
    if qmap1 != qmap:
        nc1 = build_nc(queue_map=qmap1, reps=reps)
    return nc1


def prepare_host(doc_ids, context_ids, sample_ids, paragraph_matrix,
                 word_matrix, outputs):
    doc_ids = np.asarray(doc_ids).astype(np.int64)
    context_ids = np.asarray(context_ids).astype(np.int64)
    sample_ids = np.asarray(sample_ids).astype(np.int64)
    full = np.concatenate(
        [
            np.asarray(paragraph_matrix, dtype=np.float32),
            np.asarray(word_matrix, dtype=np.float32),
            np.ascontiguousarray(np.asarray(outputs, dtype=np.float32).T),
        ],
        axis=0,
    ).astype(np.float16)

    idsA = np.concatenate(
        [doc_ids[:, None], context_ids + N_DOCS], axis=1)   # [B, 9]
    idsB = sample_ids + (N_DOCS + N_WORDS)                  # [B, 16]

    in_maps = []
    for c in range(N_CORES):
        sl = slice(c * B_CORE, (c + 1) * B_CORE)
        uqA, invA = np.unique(idsA[sl].ravel(), return_inverse=True)
        uqB, invB = np.unique(idsB[sl].ravel(), return_inverse=True)
        assert len(uqA) <= N_A and len(uqB) <= N_B
        ctab = np.zeros((CTAB_ROWS, VEC), dtype=np.float16)
        ctab[:len(uqA)] = full[uqA]
        ctab[N_A:N_A + len(uqB)] = full[uqB]
        cidA = invA.reshape(B_CORE, 9)
        cidB = invB.reshape(B_CORE, NS)

        streams = []
        for a in range(A_NG):
            blk = cidA[a * A_TPG * P:(a + 1) * A_TPG * P]
            streams.append(
                blk.reshape(A_TPG, P, 9).transpose(0, 2, 1).ravel())
        for g in range(B_NG):
            blk = cidB[g * B_TPG * P:(g + 1) * B_TPG * P]
            streams.append(
                blk.reshape(B_TPG, P, NS).transpose(0, 2, 1).ravel())
        idx = np.concatenate([_wrap16(s) for s in streams], axis=1)
        assert idx.shape == (P, IDX_COLS)
        in_maps.append({"ctab": ctab, "idx": idx})
    return in_maps


def kernel(doc_ids, context_ids, sample_ids, paragraph_matrix, word_matrix,
           outputs):
    from concourse.bass_utils import run_bass_kernel_spmd

    in_maps = prepare_host(doc_ids, context_ids, sample_ids,
                           paragraph_matrix, word_matrix, outputs)
    nc = build_nc_queued()
    out = run_bass_kernel_spmd(nc, in_maps, core_ids=list(range(N_CORES)))

    result = np.empty((B, NS), dtype=np.float32)
    for c in range(N_CORES):
        result[c * B_CORE:(c + 1) * B_CORE] = \
            out.results[c]["res"].astype(np.float32)
    return result


if __name__ == "__main__":
    pass


# revision 15
# speedup vs baseline: 13.1549x; 1.6489x over previous
"""Trainium2 Bass kernel for the word2vec-style embedding lookup problem.

reference:
    inputs = paragraph_matrix[doc_ids] + sum(word_matrix[context_ids], axis=1)
    out_cols = outputs[:, sample_ids].transpose(1, 0, 2)
    return einsum("bd,bds->bs", inputs, out_cols)

Strategy: data-parallel over the batch dim across 8 NeuronCores. The host
dedups each core's needed table rows into a per-core compact fp16 table
(standard table-sharding: rows this core touches, each exactly once):

  ctab[0:18432)      unique doc/ctx rows   (A window, int16-addressable)
  ctab[18432:51200)  unique sample columns (B window, exactly 32768 rows)

The device then does tile-aligned Q7 `dma_gather`s straight into batch
layout (stream position (k*128+p) -> batch element p of the tile):

  A: 2 gathers x 9216 rows -> [128, 8, 9, 128]; tree-add the 9 rows
     (packed-fp16 2x DVE mode) -> inputs[128, 16, 128] in SBUF
  B: 4 gathers x 8192 rows -> [128, 4, 16, 128]; mul by broadcast inputs
     + halving-add tree over d (2x mode) -> res[128, 16, 16]

fp16 everywhere (2e-2 rel-err budget; fp16 keeps us ~1e-3): 256B rows
halve HBM traffic and enable the 2x DVE modes. 51200 gathered rows/core
vs 108032 for the chunk-sorted two-stage design, no intermediate DRAM
round trip, no stream-order unpermute on the host.
"""

import numpy as np

import concourse.mybir as mybir
from concourse.bacc import Bacc
from concourse.tile import TileContext

# Problem constants (hardcoded per harness contract).
VEC = 128
N_DOCS = 100000
N_WORDS = 100000
B = 16384
CTX = 8
NS = 16
N_CORES = 8
P = 128

B_CORE = B // N_CORES            # 2048
N_TILES = B_CORE // P            # 16

# A-phase: 4 gathers of 4 tiles x 9 rows; B-phase: 8 gathers of 2 tiles x 16.
A_TPG = 4
A_NG = N_TILES // A_TPG          # 4
A_ROWS = A_TPG * 9 * P           # 4608
B_TPG = 2
B_NG = N_TILES // B_TPG          # 8
B_ROWS = B_TPG * NS * P          # 4096

N_A = B_CORE + B_CORE * CTX      # 18432 worst-case unique doc+ctx rows
N_B = B_CORE * NS                # 32768 worst-case unique sample rows
CTAB_ROWS = N_A + N_B            # 51200
IDX_COLS = (A_NG * A_ROWS + B_NG * B_ROWS) // 16  # 3200


def _wrap16(stream: np.ndarray) -> np.ndarray:
    """dma_gather index layout: j at [16k + j%16, j//16], replicated 8x."""
    assert len(stream) % 16 == 0
    arr = stream.astype(np.int16).reshape(-1, 16).T  # [16, n/16]
    return np.tile(arr, (8, 1))                      # [128, n/16]


def build_nc(queue_map=None, reps=1):
    nc = Bacc("TRN2", num_swdge_queues=4)
    f16, i16 = mybir.dt.float16, mybir.dt.int16
    ctab = nc.dram_tensor("ctab", [CTAB_ROWS, VEC], f16, kind="ExternalInput")
    idx = nc.dram_tensor("idx", [P, IDX_COLS], i16, kind="ExternalInput")
    res = nc.dram_tensor("res", [B_CORE, NS], f16, kind="ExternalOutput")

    qi = [0]

    def next_q():
        q = queue_map[qi[0] % len(queue_map)] if queue_map is not None else 0
        qi[0] += 1
        return q

    def emit_body(tc, idx_all, pools):
        a_pool, b_pool, tmp_pool, acc_pool = pools
        col = [0]

        def idx_slice(n):
            ap = idx_all[:, col[0]:col[0] + n // 16]
            col[0] += n // 16
            return ap

        inputs_all = acc_pool.tile([P, N_TILES, 1, VEC], f16, tag="inp")
        res_all = acc_pool.tile([P, N_TILES, NS], f16, tag="res")

        # ---- interleave A groups with their dependent B groups ----
        def emit_a(c):
            ct = slice(c * A_TPG, (c + 1) * A_TPG)
            t9 = a_pool.tile([P, A_TPG * 9, VEC], f16, tag="t9")
            nc.gpsimd.dma_gather(
                t9[:, :, :],
                ctab[0:N_A, :],
                idx_slice(A_ROWS),
                A_ROWS, A_ROWS, VEC,
                queue_num=next_q(), single_packet=False,
            )
            v = t9[:, :, :].rearrange("p (t r) d -> p t r d", r=9)
            s4 = tmp_pool.tile([P, A_TPG, 4, VEC], f16, tag="s4")
            nc.vector.tensor_add(
                out=s4[:, :, :, :], in0=v[:, :, 0:4, :], in1=v[:, :, 4:8, :])
            s2 = tmp_pool.tile([P, A_TPG, 2, VEC], f16, tag="s2")
            nc.vector.tensor_add(
                out=s2[:, :, :, :], in0=s4[:, :, 0:2, :], in1=s4[:, :, 2:4, :])
            s1 = tmp_pool.tile([P, A_TPG, 1, VEC], f16, tag="s1")
            nc.vector.tensor_add(
                out=s1[:, :, :, :], in0=s2[:, :, 0:1, :], in1=s2[:, :, 1:2, :])
            nc.vector.tensor_add(
                out=inputs_all[:, ct, :, :], in0=s1[:, :, :, :],
                in1=v[:, :, 8:9, :])

        def emit_b(g):
            gt = slice(g * B_TPG, (g + 1) * B_TPG)
            smp = b_pool.tile([P, B_TPG * NS, VEC], f16, tag="smp")
            nc.gpsimd.dma_gather(
                smp[:, :, :],
                ctab[N_A:CTAB_ROWS, :],
                idx_slice(B_ROWS),
                B_ROWS, B_ROWS, VEC,
                queue_num=next_q(), single_packet=False,
            )
            sv = smp[:, :, :].rearrange("p (t s) d -> p t s d", s=NS)
            nc.vector.tensor_mul(
                out=sv,
                in0=sv,
                in1=inputs_all[:, gt, :, :].to_broadcast(
                    [P, B_TPG, NS, VEC]),
            )
            # Reduce over d via in-place halving adds (2x packed-fp16 mode),
            # then one small TensorReduce over the last 4 elements.
            w = VEC
            while w > 4:
                w //= 2
                nc.vector.tensor_add(
                    out=sv[:, :, :, 0:w],
                    in0=sv[:, :, :, 0:w],
                    in1=sv[:, :, :, w:2 * w],
                )
            with nc.allow_low_precision("fp16 dot, 2e-2 rel-err budget"):
                nc.vector.reduce_sum(
                    out=res_all[:, gt, :],
                    in_=sv[:, :, :, 0:4],
                    axis=mybir.AxisListType.X,
                )

        bpg = B_NG // A_NG
        for c in range(A_NG):
            emit_a(c)
            for g in range(c * bpg, (c + 1) * bpg):
                emit_b(g)

        nc.sync.dma_start(
            out=res[:, :].rearrange("(t p) s -> p t s", p=P),
            in_=res_all[:, :, :],
        )

    with TileContext(nc) as tc:
        with (
            tc.tile_pool(name="idxp", bufs=1) as idx_pool,
            tc.tile_pool(name="ap", bufs=4) as a_pool,
            tc.tile_pool(name="bp", bufs=6) as b_pool,
            tc.tile_pool(name="tmp", bufs=4) as tmp_pool,
            tc.tile_pool(name="acc", bufs=1) as acc_pool,
        ):
            idx_all = idx_pool.tile([P, IDX_COLS], mybir.dt.int16)
            nc.sync.dma_start(out=idx_all[:, :], in_=idx[:, :])
            pools = (a_pool, b_pool, tmp_pool, acc_pool)
            for _rep in range(reps):
                emit_body(tc, idx_all, pools)

    nc.finalize()
    return nc


def gather_queue_map(nc):
    """Read each dma_gather's Tile-assigned DMASW lane; queue = lane % 4
    keeps every sem lane on a single SWDGE queue."""
    lanes = []
    for f in nc.m.functions:
        for blk in f.blocks:
            for ins in blk.instructions:
                if type(ins).__name__ == "InstDMAGatherAnt":
                    si = ins.sync_info
                    lane = None
                    for u in (si.on_update or []):
                        name = u.ant_name or ""
                        if name.startswith("DMASW"):
                            lane = int(name[5:].split("_")[0])
                    lanes.append((ins.name, lane))
    # instruction names I-k are in emission order; sort by numeric id
    lanes.sort(key=lambda t: int(t[0].split("-")[1]))
    return [(l % 4 if l is not None else 0) for _, l in lanes]


def build_nc_queued(reps=1):
    nc0 = build_nc(reps=reps)
    qmap = gather_queue_map(nc0)
    nc1 = build_nc(queue_map=qmap, reps=reps)
    qmap1 = gather_queue_map(nc1)
    if qmap1 != qmap:
        nc1 = build_nc(queue_map=qmap1, reps=reps)
    return nc1


def prepare_host(doc_ids, context_ids, sample_ids, paragraph_matrix,
                 word_matrix, outputs):
    doc_ids = np.asarray(doc_ids).astype(np.int64)
    context_ids = np.asarray(context_ids).astype(np.int64)
    sample_ids = np.asarray(sample_ids).astype(np.int64)
    full = np.concatenate(
        [
            np.asarray(paragraph_matrix, dtype=np.float32),
            np.asarray(word_matrix, dtype=np.float32),
            np.ascontiguousarray(np.asarray(outputs, dtype=np.float32).T),
        ],
        axis=0,
    ).astype(np.float16)

    idsA = np.concatenate(
        [doc_ids[:, None], context_ids + N_DOCS], axis=1)   # [B, 9]
    idsB = sample_ids + (N_DOCS + N_WORDS)                  # [B, 16]

    in_maps = []
    for c in range(N_CORES):
        sl = slice(c * B_CORE, (c + 1) * B_CORE)
        uqA, invA = np.unique(idsA[sl].ravel(), return_inverse=True)
        uqB, invB = np.unique(idsB[sl].ravel(), return_inverse=True)
        assert len(uqA) <= N_A and len(uqB) <= N_B
        ctab = np.zeros((CTAB_ROWS, VEC), dtype=np.float16)
        ctab[:len(uqA)] = full[uqA]
        ctab[N_A:N_A + len(uqB)] = full[uqB]
        cidA = invA.reshape(B_CORE, 9)
        cidB = invB.reshape(B_CORE, NS)

        # Stream packing must match emit_body's interleaved idx_slice
        # consumption order: A0, B0, B1, A1, B2, B3, ...
        streams = []
        bpg = B_NG // A_NG
        for a in range(A_NG):
            blk = cidA[a * A_TPG * P:(a + 1) * A_TPG * P]
            streams.append(
                blk.reshape(A_TPG, P, 9).transpose(0, 2, 1).ravel())
            for g in range(a * bpg, (a + 1) * bpg):
                blkb = cidB[g * B_TPG * P:(g + 1) * B_TPG * P]
                streams.append(
                    blkb.reshape(B_TPG, P, NS).transpose(0, 2, 1).ravel())
        idx = np.concatenate([_wrap16(s) for s in streams], axis=1)
        assert idx.shape == (P, IDX_COLS)
        in_maps.append({"ctab": ctab, "idx": idx})
    return in_maps


def kernel(doc_ids, context_ids, sample_ids, paragraph_matrix, word_matrix,
           outputs):
    from concourse.bass_utils import run_bass_kernel_spmd

    in_maps = prepare_host(doc_ids, context_ids, sample_ids,
                           paragraph_matrix, word_matrix, outputs)
    nc = build_nc_queued()
    out = run_bass_kernel_spmd(nc, in_maps, core_ids=list(range(N_CORES)))

    result = np.empty((B, NS), dtype=np.float32)
    for c in range(N_CORES):
        result[c * B_CORE:(c + 1) * B_CORE] = \
            out.results[c]["res"].astype(np.float32)
    return result


if __name__ == "__main__":
    pass
